# revision 46
# baseline (speedup 1.0000x reference)
"""Trainium2 Bass kernel for nn_AttnConv2d (attention-conv + dynamic conv + BN).

Math (per sample b):
  a1 = conv3x3(x, w1); a2 = conv3x3(x, w2); a3 = conv3x3(x, w3)     (SAME pad)
  attn[h,w,i,o] = sum_{p,q} a1[i,3p+h,3q+w] * a2[o,3p+h,3q+w]
  kern[o,:,:,:] = softmax(attn[.,.,.,o] / sqrt(Ci*9))
  av = conv3x3(a3, kern[b])                                         (per-sample kernel)
  y  = feature_map_stack(av)   (pure spatial/channel permutation)
  out = cm * x + NORM_SCALE * (y - mean_y) * rsqrt(var_y + eps)     (batch stats)

Sharding: data-parallel over batch, 1 sample per core, 8 cores.  The only
cross-core exchange is an AllReduce of the per-channel BN partial sums.

Implementation notes:
  - a1/a2 convs run fp8 DoubleRow matmuls with x split hi/lo: x is quantized
    host-side per-channel to an fp8 pair (hi, (x-hi)*16); each conv tap is one
    DoubleRow matmul contracting both planes with weights (wq, wq/16), giving
    ~bf16 accuracy at 2x bf16 PE throughput.  The a3 conv and the dynamic conv
    run plain-fp8 DoubleRow with taps paired two-per-instruction.
  - per-channel / per-out-channel quantization scales are folded into the fp8
    weights host-side; psum descales ride the psum->SBUF copies as
    per-partition AP scales.
  - softmax: exp bias includes ln(240) so exp output exactly fills fp8 range;
    the 1/sum normalization folds into the per-partition dynamic-conv descale.
  - attention contraction needs positions on the partition axis: conv outputs
    are written subgrid-gathered to SBUF (bf16), PE-transposed in 128-position
    chunks, then accumulated into a persistent PSUM tile.
  - a2's output-channel order is permuted host-side (channel 4*(p%32)+p//32 on
    partition p) so downstream layouts stay contiguous.
  - dynamic-conv output is stored to a DRAM scratch in pass-B-natural layout
    [p, parity, gpos, q]; the feature_map_stack permutation happens in the
    pass-C gather DMA (large contiguous runs on both sides).
  - BN group-of-4 partition sums are a tiny 0/1 matmul; pass C prefetches x
    strips during pass B / the collective wait.
"""

import os
import sys

for _p in ("/opt/trn_rl_repo", "/root/.axon_site/_ro/trn_rl_repo"):
    if os.path.isdir(_p) and _p not in sys.path:
        sys.path.insert(0, _p)
        break

import numpy as np

import concourse.bass as bass
import concourse.bacc as bacc
import concourse.tile as tile
from concourse import mybir

F32 = mybir.dt.float32
F32R = mybir.dt.float32r
BF16 = mybir.dt.bfloat16
F8 = mybir.dt.float8e4
DR = mybir.MatmulPerfMode.DoubleRow

ATTN_K = 3
NH = 2
EPS = 1e-5
NORM_SCALE = 0.1816
CI = 128
CO = 128


def _rap(base, dims, off=0):
    """Raw AP on the same tensor as `base` (keeps base's partition dim)."""
    return bass.AP(tensor=base.tensor, offset=base.offset + off,
                   ap=[base.ap[0]] + [list(d) for d in dims])


def build_nc(H, W, R, n_cores, cm, a12_mode="fp8a1", level=5):
    """Build the per-core Bass kernel. R = strip rows (div by 6, even).

    a12_mode: 'hilo' (fp8 DoubleRow w/ hi-lo x) or 'bf16'.
    """
    assert H % R == 0 and R % 6 == 0 and W % 6 == 0
    NS = H // R                      # strips
    Wq = W // 3                      # attn subgrid cols
    P = (R // 3) * Wq                # attn positions per offset per strip
    S = H // 2                       # quadrant size of feature_map_stack
    NT = R // 2                      # psum tiles (2 rows) per strip
    PQ = (R // 2) * (W // 2)         # parity-split positions per strip
    W2 = W + 2                       # padded row width
    PL = (H + 2) * W2                # fp8 plane stride
    N_TOT = float(n_cores * H * W)   # BN count per channel
    SCL = 1.0 / float(np.sqrt(CI * 9))
    LN240 = float(np.log(240.0))
    NPLANES = 2 if a12_mode == "hilo" else 1

    nc = bacc.Bacc("TRN2", target_bir_lowering=False, debug=False,
                   num_devices=n_cores)

    xbf_in = nc.dram_tensor("xbf", [128, H + 2, W2], BF16,
                            kind="ExternalInput").ap()   # host-padded (+1 ring)
    xr_in = nc.dram_tensor("xr", [128, H, W], BF16,
                           kind="ExternalInput").ap()    # unpadded, pass C
    xq_in = nc.dram_tensor("xq", [128, NPLANES, H + 2, W2], F8,
                           kind="ExternalInput").ap()
    if a12_mode == "hilo":
        w1_in = nc.dram_tensor("w1q", [128, 9, 2, 128], F8,
                               kind="ExternalInput").ap()
        w2_in = nc.dram_tensor("w2q", [128, 9, 2, 128], F8,
                               kind="ExternalInput").ap()
    elif a12_mode == "fp8a1":
        w1_in = nc.dram_tensor("w1q", [128, 9, 128], F8,
                               kind="ExternalInput").ap()
        w2_in = nc.dram_tensor("w2t", [128, 9, 128], BF16,
                               kind="ExternalInput").ap()
    else:
        w1_in = nc.dram_tensor("w1t", [128, 9, 128], BF16,
                               kind="ExternalInput").ap()
        w2_in = nc.dram_tensor("w2t", [128, 9, 128], BF16,
                               kind="ExternalInput").ap()
    w3_in = nc.dram_tensor("w3q", [128, 9, 128], F8, kind="ExternalInput").ap()
    id_in = nc.dram_tensor("ident", [128, 128], BF16, kind="ExternalInput").ap()
    idq_in = nc.dram_tensor("idq", [128, 128], F8, kind="ExternalInput").ap()
    gp_in = nc.dram_tensor("gsum", [128, 128], F32, kind="ExternalInput").ap()
    mk_in = nc.dram_tensor("mask4", [128, 4], F32, kind="ExternalInput").ap()
    sv_in = nc.dram_tensor("svec", [128, 4], F32, kind="ExternalInput").ap()
    out_d = nc.dram_tensor("out", [128, H, W], BF16,
                           kind="ExternalOutput").ap()
    # DRAM scratch for the second image half only (first half moves via
    # SBUF->SBUF gathers during pass B): [p, parity, gpos-48, q]
    avp_d = nc.dram_tensor("avp", [128, 4, H // 4, W // 2], F8).ap()

    with tile.TileContext(nc) as tc:
        consts = tc.alloc_tile_pool(name="consts", bufs=1)
        if a12_mode == "hilo":
            w1t = consts.tile([128, 9, 2, 128], F8, tag="w1t")
            w2t = consts.tile([128, 9, 2, 128], F8, tag="w2t")
        elif a12_mode == "fp8a1":
            w1t = consts.tile([128, 9, 128], F8, tag="w1t")
            w2t = consts.tile([128, 9, 128], BF16, tag="w2t")
        else:
            w1t = consts.tile([128, 9, 128], BF16, tag="w1t")
            w2t = consts.tile([128, 9, 128], BF16, tag="w2t")
        w3t = consts.tile([128, 9, 128], F8, tag="w3t")
        ident = consts.tile([128, 128], BF16, tag="ident")
        idq = consts.tile([128, 128], F8, tag="idq")
        gsum = consts.tile([128, 128], F32, tag="gsum")
        mask4 = consts.tile([128, 4], F32, tag="mask4")
        svec = consts.tile([128, 4], F32, tag="svec")
        if a12_mode == "hilo":
            nc.sync.dma_start(out=w1t[:], in_=w1_in[:])
            nc.sync.dma_start(out=w2t[:], in_=w2_in[:])
        else:
            nc.sync.dma_start(out=w1t[:], in_=w1_in[:])
            nc.sync.dma_start(out=w2t[:], in_=w2_in[:])
        nc.sync.dma_start(out=w3t[:], in_=w3_in[:])
        nc.sync.dma_start(out=ident[:], in_=id_in[:])
        nc.sync.dma_start(out=idq[:], in_=idq_in[:])
        nc.sync.dma_start(out=gsum[:], in_=gp_in[:])
        nc.sync.dma_start(out=mask4[:], in_=mk_in[:])
        nc.sync.dma_start(out=svec[:], in_=sv_in[:])

        cc_pool = tc.alloc_tile_pool(name="ccd", bufs=1, space="DRAM")
        warm_in = cc_pool.tile([128, 1], F32, tag="warm_in")
        warm_out = cc_pool.tile([128, 1], F32, tag="warm_out")
        cc_in = cc_pool.tile([128, 8], F32, tag="cc_in")
        cc_out = cc_pool.tile([128, 8], F32, tag="cc_out")
        nc.gpsimd.dma_start(out=warm_in[:], in_=svec[:, 0:1])
        nc.gpsimd.collective_compute(
            "AllReduce", mybir.AluOpType.add,
            replica_groups=[list(range(n_cores))],
            ins=[warm_in.opt()], outs=[warm_out.opt()])

        small = tc.alloc_tile_pool(name="small", bufs=1)
        stats_cols = small.tile([128, NS, 4, 2], F32, tag="stats_cols")
        sloc = small.tile([128, 8], F32, tag="sloc")
        sglob = small.tile([128, 8], F32, tag="sglob")
        scalars = small.tile([128, 16], F32, tag="scalars")
        msb = small.tile([128, 8], F32, tag="msb")
        sel = small.tile([128, 4], F32, tag="sel")

        kern_pool = tc.alloc_tile_pool(name="kern", bufs=1)
        kernP = [kern_pool.tile([128, 2, 128], F8, tag=f"kP{k}", name=f"kP{k}")
                 for k in range(4)]
        kernS = kern_pool.tile([128, 128], F8, tag="kS")

        # pass-C x prefetch pool (loads issued inside pass B's strip loop);
        # allocated early so it can outlive the pass A/B pools (LIFO stack)
        pc_x = tc.alloc_tile_pool(name="pc_x", bufs=1)
        x_strips = [None] * NS
        # pass-C consumption order matches av_s production order (pass-B strip
        # ss feeds pass-C strips ss//2 and ss//2+4)
        c_order = [v for m in range(NS // 2) for v in (m, m + NS // 2)]
        pc_a = tc.alloc_tile_pool(name="pc_a", bufs=4)
        av_s_tiles = [None] * NS

        def load_x_strip(s):
            xt = pc_x.tile([128, R, W], BF16, tag="x_s")
            nc.sync.dma_start(out=xt[:], in_=xr_in[:, s * R:s * R + R, :])
            return xt

        a3_pool = tc.alloc_tile_pool(name="a3p", bufs=1)
        a3p = a3_pool.tile([128, H + 2, W2], F8, tag="a3p")
        # zero the pad border of a3p once
        nc.vector.memset(_rap(a3p[:], [[1, W2]]), 0.0)                       # row 0
        nc.vector.memset(_rap(a3p[:], [[1, W2]], (H + 1) * W2), 0.0)         # row H+1
        nc.vector.memset(_rap(a3p[:], [[W2, H + 2]]), 0.0)                   # col 0
        nc.vector.memset(_rap(a3p[:], [[W2, H + 2]], W + 1), 0.0)            # col W+1

        # full-image fp8 x in two overlapping row-halves (padded rows 0..97,
        # 96..193) so the second half's load overlaps the first half's compute
        HH = R + 2
        xq_pool = tc.alloc_tile_pool(name="xqp", bufs=2)

        def load_xq_strip(s):
            xt = xq_pool.tile([128, NPLANES, HH, W2], F8, tag="xqs")
            nc.sync.dma_start(out=xt[:],
                              in_=xq_in[:, :, s * R:s * R + HH, :])
            return xt

        attn_psp = tc.alloc_tile_pool(name="attn_ps", bufs=1, space="PSUM")
        attn_ps = attn_psp.tile([128, 9 * 128], F32, tag="attn")

        # ---------------- pass A: static convs + attention accumulation ------
        pa_x = tc.alloc_tile_pool(name="pa_x", bufs=2)
        pa_g = tc.alloc_tile_pool(name="pa_g", bufs=2)
        pa_t = tc.alloc_tile_pool(name="pa_t", bufs=2)
        pa_cps = tc.alloc_tile_pool(name="pa_cps", bufs=3, space="PSUM")
        pa_tps = tc.alloc_tile_pool(name="pa_tps", bufs=2, space="PSUM")

        # strip-0 x load first so the first conv isn't queued behind the
        # full-image fp8 loads
        xq_pre = load_xq_strip(0)
        xs_pre = None
        if a12_mode in ("bf16", "fp8a1"):
            xs_pre = pa_x.tile([128, R + 2, W2], BF16, tag="xs")
            nc.sync.dma_start(out=xs_pre[:], in_=xbf_in[:, 0:R + 2, :])

        for s in range(NS):
            y0 = s * R
            xh = xq_pre if s == 0 else load_xq_strip(s)
            yb = 0                                # strip base row within tile
            if a12_mode in ("bf16", "fp8a1"):
                if s == 0:
                    xs = xs_pre
                else:
                    xs = pa_x.tile([128, R + 2, W2], BF16, tag="xs")
                    nc.sync.dma_start(out=xs[:],
                                      in_=xbf_in[:, y0:y0 + R + 2, :])

            a1g = pa_g.tile([128, 9, P], BF16, tag="a1g")
            a2g = pa_g.tile([128, 9, P], BF16, tag="a2g")
            for ci, (wt, gdst, sc_col) in enumerate(
                    ((w1t, a1g, 2), (w2t, a2g, 3))):
                cmode = a12_mode
                if a12_mode == "fp8a1":
                    cmode = "fp8" if ci == 0 else "bf16"
                for t in range(NT):
                    cps = pa_cps.tile([128, 2 * W], F32, tag="cps")
                    if cmode == "fp8":
                        for k in range(3):
                            rhs = _rap(xh[:], [[W2, 2], [W2, 2], [1, W]],
                                       (yb + 2 * t) * W2 + k)
                            nc.tensor.matmul(
                                cps[:, :],
                                _rap(wt[:], [[3 * 128, 2], [1, 128]], k * 128),
                                rhs, start=(k == 0), stop=False, perf_mode=DR)
                        rhs = _rap(xh[:], [[1, 2], [W2, 2], [1, W]],
                                   (yb + 2 * t + 2) * W2)
                        nc.tensor.matmul(
                            cps[:, :], _rap(wt[:], [[128, 2], [1, 128]],
                                            6 * 128),
                            rhs, start=False, stop=False, perf_mode=DR)
                        rhs = _rap(xh[:], [[W2, 2], [1, W]],
                                   (yb + 2 * t + 2) * W2 + 2)
                        nc.tensor.matmul(cps[:, :], wt[:, 8, :], rhs,
                                         start=False, stop=True)
                    elif cmode == "hilo":
                        for k in range(9):
                            dy, dx = divmod(k, 3)
                            rhs = _rap(xh[:], [[HH * W2, 2], [W2, 2], [1, W]],
                                       (yb + 2 * t + dy) * W2 + dx)
                            nc.tensor.matmul(
                                cps[:, :],
                                _rap(wt[:], [[128, 2], [1, 128]], k * 256),
                                rhs, start=(k == 0), stop=(k == 8),
                                perf_mode=DR)
                    else:
                        for k in range(9):
                            dy, dx = divmod(k, 3)
                            rhs = xs[:, 2 * t + dy:2 * t + dy + 2, dx:dx + W]
                            nc.tensor.matmul(cps[:, :], wt[:, k, :], rhs,
                                             start=(k == 0), stop=(k == 8))
                    # scatter rows (2t, 2t+1) into subgrid-major layout
                    ya, yb2 = 2 * t, 2 * t + 1
                    ha, ra = ya % 3, ya // 3
                    hb, rb = yb2 % 3, yb2 // 3
                    offa = (3 * ha) * P + ra * Wq
                    sd = (3 * hb) * P + rb * Wq - offa
                    dst = _rap(gdst[:], [[sd, 2], [P, 3], [1, Wq]], offa)
                    src = _rap(cps[:], [[W, 2], [1, 3], [3, Wq]])
                    if cmode in ("hilo", "fp8"):
                        nc.scalar.mul(dst, src, svec[:, sc_col:sc_col + 1])
                    else:
                        nc.scalar.copy(out=dst, in_=src)
            # a3 conv: plain-fp8 DoubleRow, taps paired (k,k+3) + (6,7) + 8
            for t in range(NT):
                cps = pa_cps.tile([128, 2 * W], F32, tag="cps")
                for k in range(3):
                    rhs = _rap(xh[:], [[W2, 2], [W2, 2], [1, W]],
                               (yb + 2 * t) * W2 + k)
                    nc.tensor.matmul(
                        cps[:, :], _rap(w3t[:], [[3 * 128, 2], [1, 128]],
                                        k * 128),
                        rhs, start=(k == 0), stop=False, perf_mode=DR)
                rhs = _rap(xh[:], [[1, 2], [W2, 2], [1, W]],
                           (yb + 2 * t + 2) * W2)
                nc.tensor.matmul(
                    cps[:, :], _rap(w3t[:], [[128, 2], [1, 128]], 6 * 128),
                    rhs, start=False, stop=False, perf_mode=DR)
                rhs = _rap(xh[:], [[W2, 2], [1, W]], (yb + 2 * t + 2) * W2 + 2)
                nc.tensor.matmul(cps[:, :], w3t[:, 8, :], rhs,
                                 start=False, stop=True)
                nc.vector.tensor_scalar_mul(
                    a3p[:, 1 + y0 + 2 * t:1 + y0 + 2 * t + 2, 1:1 + W],
                    _rap(cps[:], [[W, 2], [1, W]]), svec[:, 0:1])
            # attention: transpose chunks and accumulate
            for hw in range(9):
                for c0 in range(0, P, 128):
                    ch = min(128, P - c0)
                    t1 = pa_tps.tile([128, 128], BF16, tag="tps")
                    nc.tensor.transpose(t1[0:ch, :], a1g[:, hw, c0:c0 + ch], ident[:])
                    a1T = pa_t.tile([128, 128], BF16, tag="aT")
                    nc.vector.tensor_copy(a1T[0:ch, :], t1[0:ch, :])
                    t2 = pa_tps.tile([128, 128], BF16, tag="tps")
                    nc.tensor.transpose(t2[0:ch, :], a2g[:, hw, c0:c0 + ch], ident[:])
                    a2T = pa_t.tile([128, 128], BF16, tag="aT")
                    nc.vector.tensor_copy(a2T[0:ch, :], t2[0:ch, :])
                    nc.tensor.matmul(
                        attn_ps[:, hw * 128:(hw + 1) * 128],
                        a2T[0:ch, :], a1T[0:ch, :],
                        start=(s == 0 and c0 == 0 and hw in (0, 4, 8)),
                        stop=(s == NS - 1 and c0 + 128 >= P and hw in (3, 7, 8)),
                        skip_group_check=True)

        pa_tps.release(); pa_cps.release()
        pa_t.release(); pa_g.release(); pa_x.release()
        xq_pool.release()
        pc_x2 = tc.alloc_tile_pool(name="pc_x2", bufs=NS - 3)
        pc_a2 = tc.alloc_tile_pool(name="pc_a2", bufs=3)

        def load_x_strip2(s):
            xt = pc_x2.tile([128, R, W], BF16, tag="x_s2")
            nc.sync.dma_start(out=xt[:], in_=xr_in[:, s * R:s * R + R, :])
            return xt

        # ---------------- softmax + kern transposes -------------------------
        if level >= 2:
            sm_pool = tc.alloc_tile_pool(name="smx", bufs=1)
            attn_sb = sm_pool.tile([128, 9 * 128], F32, tag="attn_sb")
            nc.vector.tensor_copy(attn_sb[:], attn_ps[:])
            attn_psp.release()
            k_tps = tc.alloc_tile_pool(name="k_tps", bufs=2, space="PSUM")
            mx = scalars[:, 0:1]
            nmx = scalars[:, 1:2]
            ssum = scalars[:, 2:3]
            rsum = scalars[:, 3:4]
            avscl = scalars[:, 14:15]
            nc.vector.reduce_max(mx, attn_sb[:], axis=mybir.AxisListType.X)
            # exp bias includes ln(240): esb = 240*exp((attn-mx)*SCL), max 240
            nc.vector.tensor_scalar(nmx, mx, -SCL, LN240,
                                    op0=mybir.AluOpType.mult,
                                    op1=mybir.AluOpType.add)
            esb = sm_pool.tile([128, 9 * 128], BF16, tag="esb")
            nc.scalar.activation(esb[:], attn_sb[:],
                                 mybir.ActivationFunctionType.Exp,
                                 bias=nmx, scale=SCL)
            nc.vector.reduce_sum(ssum, esb[:], axis=mybir.AxisListType.X)
            nc.vector.reciprocal(rsum, ssum)
            # dynamic-conv descale: rsum (softmax norm) * C3/240 (fp8 scales)
            nc.vector.tensor_mul(avscl, rsum, svec[:, 1:2])
            for hw in range(9):
                tp = k_tps.tile([128, 128], BF16, tag="ktp")
                nc.tensor.transpose(tp[:], esb[:, hw * 128:(hw + 1) * 128],
                                    ident[:])
                if hw < 6:
                    dst = kernP[hw % 3][:, hw // 3, :]
                elif hw < 8:
                    dst = kernP[3][:, hw - 6, :]
                else:
                    dst = kernS[:]
                nc.vector.tensor_copy(dst, tp[:])
            k_tps.release(); sm_pool.release()
        else:
            attn_psp.release()

        # ---------------- pass B: dynamic conv + stats + natural store ------
        if level >= 3:
            pb_av = tc.alloc_tile_pool(name="pb_av", bufs=4)
            pb_sq = tc.alloc_tile_pool(name="pb_sq", bufs=2)
            pb_cps = tc.alloc_tile_pool(name="pb_cps", bufs=5, space="PSUM")
            for s in range(NS):
                y0 = s * R
                # av parity-split: av_sp[c, 2i+j, p, q] = av[c, 2p+i, 2q+j]
                av_sp = pb_av.tile([128, 4, R // 2, W // 2], F8, tag="av")
                for t in range(NT):
                    cps = pb_cps.tile([128, 2 * W], F32, tag="cps2")
                    yb = y0 + 2 * t
                    for k in range(3):
                        rhs = _rap(a3p[:], [[W2, 2], [W2, 2], [1, W]],
                                   yb * W2 + k)
                        nc.tensor.matmul(cps[:, :], kernP[k][:], rhs,
                                         start=(k == 0), stop=False,
                                         perf_mode=DR)
                    rhs = _rap(a3p[:], [[1, 2], [W2, 2], [1, W]],
                               (yb + 2) * W2)
                    nc.tensor.matmul(cps[:, :], kernP[3][:], rhs,
                                     start=False, stop=False, perf_mode=DR)
                    rhs = _rap(a3p[:], [[W2, 2], [1, W]], (yb + 2) * W2 + 2)
                    nc.tensor.matmul(cps[:, :], kernS[:], rhs,
                                     start=False, stop=True)
                    nc.scalar.mul(
                        _rap(av_sp[:], [[2 * PQ, 2], [PQ, 2], [1, W // 2]],
                             t * (W // 2)),
                        _rap(cps[:], [[W, 2], [1, 2], [2, W // 2]]),
                        avscl)
                nc.vector.reduce_sum(
                    _rap(stats_cols[:], [[2, 4]], s * 8),
                    _rap(av_sp[:], [[PQ, 4], [1, PQ]]),
                    axis=mybir.AxisListType.X)
                for pi in range(4):
                    psrc = _rap(av_sp[:], [[1, PQ]], pi * PQ)
                    sq = pb_sq.tile([128, PQ], F8, tag="sq")
                    if pi < 2:
                        nc.scalar.activation(
                            out=sq[:], in_=psrc,
                            func=mybir.ActivationFunctionType.Square,
                            accum_out=stats_cols[:, s, pi, 1:2])
                    else:
                        nc.vector.tensor_mul(sq[:], psrc, psrc)
                        nc.vector.reduce_sum(stats_cols[:, s, pi, 1:2], sq[:],
                                             axis=mybir.AxisListType.X)
                # feature_map_stack: av_sp[32*(2Yb+bh)+q2, piv, gp, q]
                # -> av_s(sc)[4*q2+piv, bh].  First image half: SBUF->SBUF
                # gathers now (prefetch); second half: DRAM store, gathered
                # lazily in pass C.
                half = s % 2
                if s < NS // 2:
                    for sc in (s // 2, s // 2 + NS // 2):
                        Yb = sc // (NS // 2)
                        if half == 0:
                            av_s_tiles[sc] = pc_a.tile(
                                [128, 2, R, W // 2], F8, tag="av_s",
                                name=f"av_s{sc}")
                        avt = av_s_tiles[sc]
                        for bh in range(2):
                            for piv in range(4):
                                dst = avt[piv:128:4, bh,
                                          half * (R // 2):
                                          (half + 1) * (R // 2), :]
                                grp = 32 * (2 * Yb + bh)
                                nc.sync.dma_start(
                                    out=dst, in_=av_sp[grp:grp + 32, piv, :, :])
                else:
                    nc.sync.dma_start(
                        out=avp_d[:, :, (s - NS // 2) * (R // 2):
                                  (s - NS // 2 + 1) * (R // 2), :],
                        in_=av_sp[:])
                if s < 1:
                    x_strips[c_order[s]] = load_x_strip(c_order[s])
                else:
                    x_strips[c_order[s]] = load_x_strip2(c_order[s])
            pb_cps.release()
            pb_sq.release(); pb_av.release()

        # ---------------- AllReduce of BN partial sums ----------------------
        if level >= 4:
            nc.vector.reduce_sum(
                _rap(sloc[:], [[2, 4], [1, 2]]),
                _rap(stats_cols[:], [[2, 4], [1, 2], [8, NS]]),
                axis=mybir.AxisListType.X)
            nc.gpsimd.dma_start(out=cc_in[:], in_=sloc[:])
            nc.gpsimd.collective_compute(
                "AllReduce", mybir.AluOpType.add,
                replica_groups=[list(range(n_cores))],
                ins=[cc_in.opt()], outs=[cc_out.opt()])
            nc.gpsimd.dma_start(out=sglob[:], in_=cc_out[:])

            # ------------ BN coefficients (per out-channel) -----------------
            bn_ps = tc.alloc_tile_pool(name="bn_ps", bufs=1, space="PSUM")
            gps = bn_ps.tile([128, 8], F32, tag="gps")
            nc.tensor.matmul(gps[:], gsum[:], sglob[:], start=True, stop=True)
            nc.vector.tensor_copy(msb[:], gps[:])
            bn_ps.release()
            mean = scalars[:, 4:5]
            e2 = scalars[:, 5:6]
            msq = scalars[:, 6:7]
            var = scalars[:, 7:8]
            sd = scalars[:, 8:9]
            rstd = scalars[:, 9:10]
            sc = scalars[:, 10:11]
            bb0 = scalars[:, 11:12]
            bb = scalars[:, 12:13]
            nc.vector.tensor_mul(sel[:], _rap(msb[:], [[2, 4]]), mask4[:])
            nc.vector.reduce_sum(mean, sel[:], axis=mybir.AxisListType.X)
            nc.vector.tensor_scalar_mul(mean, mean, 1.0 / N_TOT)
            nc.vector.tensor_mul(sel[:], _rap(msb[:], [[2, 4]], 1), mask4[:])
            nc.vector.reduce_sum(e2, sel[:], axis=mybir.AxisListType.X)
            nc.vector.tensor_scalar_mul(e2, e2, 1.0 / N_TOT)
            nc.vector.tensor_mul(msq, mean, mean)
            nc.vector.tensor_tensor(out=var, in0=e2, in1=msq,
                                    op=mybir.AluOpType.subtract)
            eps_ap = scalars[:, 13:14]
            nc.vector.memset(eps_ap, EPS)
            nc.scalar.activation(sd, var, mybir.ActivationFunctionType.Sqrt,
                                 bias=eps_ap)
            nc.vector.reciprocal(rstd, sd)
            nc.vector.tensor_scalar_mul(sc, rstd, NORM_SCALE)
            nc.vector.tensor_mul(bb0, mean, sc)
            nc.vector.tensor_scalar_mul(bb, bb0, -1.0)

        # ---------------- pass C: out = cm*x + sc*avp + bb ------------------
        # out partition C reads av channel at avp partition 32*(2*Yb+bh)+C//4,
        # section pi=C%4 (the feature_map_stack gather, done by the load DMA).
        if level >= 5:
            pc_t = tc.alloc_tile_pool(name="pc_t", bufs=1)
            pc_o = tc.alloc_tile_pool(name="pc_o", bufs=2)
            CHP = 4 * (H // 4) * (W // 2)

            def dram_gather(s, pool):
                m, Yb = s % (NS // 2), s // (NS // 2)
                gp0 = m * R - (NS // 2) * (R // 2)
                avt = pool.tile([128, 2, R, W // 2], F8, tag="av_s2",
                                name=f"av_s{s}")
                for bh in range(2):
                    for piv in range(4):
                        src_ap = bass.AP(
                            tensor=avp_d.tensor,
                            offset=(32 * (2 * Yb + bh)) * CHP
                            + piv * (H // 4) * (W // 2) + gp0 * (W // 2),
                            ap=[[CHP, 32], [W // 2, R], [1, W // 2]])
                        nc.sync.dma_start(out=avt[piv:128:4, bh], in_=src_ap)
                av_s_tiles[s] = avt

            # these run during the collective wait (fresh buffers, no blocking)
            for s in (2, 6, 3):
                dram_gather(s, pc_a2)
            for s in c_order:
                y0 = s * R
                x_s = x_strips[s] if x_strips[s] is not None else load_x_strip(s)
                if av_s_tiles[s] is None:
                    dram_gather(s, pc_a)
                av_s = av_s_tiles[s]
                t_s = pc_t.tile([128, R * W], BF16, tag="t_s")
                nc.vector.tensor_scalar(
                    out=t_s[:],
                    in0=_rap(av_s[:], [[W // 2, R], [R * W // 2, 2], [1, W // 2]]),
                    scalar1=sc, scalar2=bb, op0=mybir.AluOpType.mult,
                    op1=mybir.AluOpType.add)
                x_ap = x_s[:]
                o_s = pc_o.tile([128, R * W], BF16, tag="o_s")
                if cm == 1.0:
                    nc.vector.tensor_add(o_s[:], t_s[:], x_ap)
                else:
                    xc = pc_t.tile([128, R * W], BF16, tag="xc")
                    nc.scalar.mul(xc[:], x_ap, float(cm))
                    nc.vector.tensor_add(o_s[:], t_s[:], xc[:])
                nc.sync.dma_start(out=out_d[:, y0:y0 + R, :], in_=o_s[:])
            pc_o.release(); pc_t.release()
        pc_a2.release()
        pc_x2.release()
        a3_pool.release()
        pc_a.release()
        pc_x.release()
        kern_pool.release()

        small.release()
        cc_pool.release()
        consts.release()

    nc.compile()
    return nc


def _wt_layout(w, permute_out=False):
    """[Co,Ci,3,3] -> lhsT layout [Ci, 9, Co] (optionally out-chan permuted)."""
    wt = np.ascontiguousarray(w.transpose(1, 2, 3, 0).reshape(128, 9, 128))
    if permute_out:
        p = np.arange(128)
        co_of_p = 4 * (p % 32) + p // 32     # partition p holds channel co_of_p
        wt = np.ascontiguousarray(wt[:, :, co_of_p])
    return wt


def _f8(v):
    import ml_dtypes
    return np.clip(v, -240.0, 240.0).astype(ml_dtypes.float8_e4m3)


def make_in_maps(x, w1, w2, w3, a12_mode="fp8a1"):
    """Per-sample input dicts: fp8-quantized x/w with folded scales."""
    import ml_dtypes
    x = np.asarray(x, np.float32)
    B = x.shape[0]
    w1 = np.asarray(w1, np.float32)
    w2 = np.asarray(w2, np.float32)
    w3 = np.asarray(w3, np.float32)
    w1t = _wt_layout(w1)
    w2t = _wt_layout(w2, permute_out=True)
    w3t = _wt_layout(w3)
    C3 = 6.0 * float(np.linalg.norm(w3.reshape(128, -1), axis=1).max())

    ident_bf = np.eye(128, dtype=np.float32).astype(ml_dtypes.bfloat16)
    idq = np.eye(128, dtype=np.float32).astype(ml_dtypes.float8_e4m3)
    p = np.arange(128)
    gsum = (p[:, None] % 32 == p[None, :] // 4).astype(np.float32)
    mask4 = (p[:, None] % 4 == np.arange(4)[None, :]).astype(np.float32)
    common = {"ident": ident_bf, "idq": idq, "gsum": gsum, "mask4": mask4}

    maps = []
    for b in range(B):
        xb = x[b]
        A = np.maximum(np.abs(xb).max(axis=(1, 2)), 1e-30)       # [Ci]
        xs = xb * (240.0 / A)[:, None, None]
        xq_hi = _f8(xs)
        svec = np.zeros((128, 4), np.float32)
        svec[:, 1] = C3 / 240.0

        def quant_w(wt):
            ws = wt * (A / 240.0)[:, None, None]
            B_o = np.maximum(np.abs(ws).max(axis=(0, 1)), 1e-30)  # [Co]
            wq = _f8(ws * (240.0 / B_o)[None, None, :])
            return wq, B_o

        w3q, B3 = quant_w(w3t)
        svec[:, 0] = (B3 / C3).astype(np.float32)
        m = dict(common)
        if a12_mode == "hilo":
            xq_lo = _f8((xs - xq_hi.astype(np.float32)) * 16.0)
            xq = np.stack([xq_hi, xq_lo], axis=1)                # [128,2,H,W]
            w1q, B1 = quant_w(w1t)
            w2q, B2 = quant_w(w2t)
            svec[:, 2] = (B1 / 240.0).astype(np.float32)
            svec[:, 3] = (B2 / 240.0).astype(np.float32)
            m["w1q"] = np.stack(
                [w1q, _f8(w1q.astype(np.float32) / 16.0)], axis=2)
            m["w2q"] = np.stack(
                [w2q, _f8(w2q.astype(np.float32) / 16.0)], axis=2)
        elif a12_mode == "fp8a1":
            xq = xq_hi[:, None]
            w1q, B1 = quant_w(w1t)
            svec[:, 2] = (B1 / 240.0).astype(np.float32)
            m["w1q"] = w1q
            m["w2t"] = np.ascontiguousarray(w2t).astype(ml_dtypes.bfloat16)
        else:
            xq = xq_hi[:, None]
            m["w1t"] = np.ascontiguousarray(w1t).astype(ml_dtypes.bfloat16)
            m["w2t"] = np.ascontiguousarray(w2t).astype(ml_dtypes.bfloat16)
        xb_bf = xb.astype(ml_dtypes.bfloat16)
        m["xbf"] = np.pad(xb_bf, ((0, 0), (1, 1), (1, 1)))
        m["xr"] = xb_bf
        m["xq"] = np.pad(xq, ((0, 0), (0, 0), (1, 1), (1, 1)))
        m["w3q"] = w3q
        m["svec"] = svec
        maps.append(m)
    return maps


_CACHE = {}


def kernel(x, w1, w2, w3, conv_momentum):
    from concourse.bass_utils import run_bass_kernel_spmd

    x = np.asarray(x, np.float32)
    B, Ci, H, W = x.shape
    cm = float(np.asarray(conv_momentum))
    key = (H, W, B, cm)
    if key not in _CACHE:
        _CACHE[key] = build_nc(H, W, 24, B, cm)
    nc = _CACHE[key]
    in_maps = make_in_maps(x, w1, w2, w3)
    res = run_bass_kernel_spmd(nc, in_maps, list(range(B)))
    out = np.stack(
        [np.asarray(res.results[b]["out"]).astype(np.float32).reshape(128, H, W)
         for b in range(B)], axis=0)
    return out
